# revision 36
# baseline (speedup 1.0000x reference)
"""DeeperGCN (GENConv softmax-aggr + virtual node) on 8 Trainium2 NeuronCores.

Self-contained kernel: host-side index preprocessing (graph partitioning /
slot layout only), one static SPMD Bass/Tile program compiled for 8 cores,
executed via concourse.bass_utils.run_bass_kernel_spmd.

Distribution (static program, no data-dependent control flow):
  - nodes sharded contiguously; core c owns original nodes [6250c, 6250(c+1)),
    padded to NLOC=6400 (NBLK=50 blocks x 128).
  - edges partitioned by dst owner, grouped per (block, src-table-half) into
    fixed tile slots; slot -> (tile, partition) = (slot//128, slot%128).
  - per layer: feed vectors AllGathered into an fp16 HBM table [NPAD, 64];
    per-edge source rows fetched with dma_gather (256B descriptors via the
    elem_step=64 two-row trick; int16 indices fit because each table half has
    NPAD/2 = 25600 rows); messages exp/weighted on ACT/DVE; per-dst-block
    softmax sums via one-hot matmuls (S^T fp8, device-built once) into PSUM.
  - virtual-node pooled embeddings [G, 64] AllReduced per layer; vn[batch]
    re-expansion also via one-hot matmuls.
"""
import sys

sys.path.insert(0, "/opt/trn_rl_repo")

import os
import numpy as np
import ml_dtypes
EM_DT_ENV = os.environ.get("K_EM16", "0") == "1"

import concourse.bass as bass
import concourse.bacc as bacc
import concourse.tile as tile
import concourse.mybir as mybir
from concourse.tile_rust import add_dep_helper
from concourse.masks import make_identity
import dataclasses


def _two_row_view(ap, rows, two_d):
    """Overlapping-row AP view: row stride D, row length 2D (gather trick)."""
    return dataclasses.replace(ap, ap=type(ap.ap)([[two_d // 2, rows],
                                                   [1, two_d]]))

# problem constants
N, E, D, G_FULL, L = 50000, 400000, 64, 256, 4
MSG_EPS = 1e-7
LN_EPS = 1e-5
NC_ = 8
P = 128

FP16, FP8, BF16, F32, I16 = (mybir.dt.float16, mybir.dt.float8e4,
                             mybir.dt.bfloat16, mybir.dt.float32,
                             mybir.dt.int16)
NP_FP16, NP_FP8, NP_BF16 = np.float16, ml_dtypes.float8_e4m3, ml_dtypes.bfloat16


class CFG:
    def __init__(self, n, e, g, nblk, tblk_h, chunk_blks):
        self.N, self.E, self.G = n, e, g
        self.NBLK, self.TBLK_H = nblk, tblk_h
        self.CAP_H = tblk_h * P
        self.NLOC = nblk * P
        self.NPAD = self.NLOC * NC_
        self.HALF = self.NPAD // 2
        self.NLOC_REAL = n // NC_
        self.SLOTS_H = nblk * self.CAP_H
        self.NTILE_H = nblk * tblk_h
        self.NTILE = 2 * self.NTILE_H
        self.CHUNK_BLKS = chunk_blks
        assert nblk % chunk_blks == 0
        self.NCHUNK = nblk // chunk_blks
        self.CH_TILES = chunk_blks * tblk_h
        self.CH_IDX = self.CH_TILES * P
        self.GT = max(1, g // P)

    @staticmethod
    def full():
        return CFG(N, E, G_FULL, 50, 5, int(os.environ.get('K_CB', '2')))

    @staticmethod
    def small():
        return CFG(8192, 24576, 64, 8, 2, 4)


# ---------------- host-side layout (pure index work) ----------------

def build_layout(cfg, edge_index, edge_attr, batch):
    src = np.asarray(edge_index[0], np.int64)
    dst = np.asarray(edge_index[1], np.int64)
    batch = np.asarray(batch, np.int64)
    ea = np.asarray(edge_attr, np.int64)
    etype_all = ea[:, 0] * 64 + ea[:, 1] * 8 + ea[:, 2]

    nr, nl = cfg.NLOC_REAL, cfg.NLOC
    c_of = src // nr
    gsrc = nl * c_of + (src - nr * c_of)
    owner = dst // nr

    def wrap16(lin):
        w = np.zeros((P, len(lin) // 16), np.int16)
        cols = np.arange(len(lin)) // 16
        rows = np.arange(len(lin)) % 16
        for r in range(8):
            w[rows + 16 * r, cols] = lin.astype(np.int16)
        return w

    cores = []
    for c in range(NC_):
        em = np.nonzero(owner == c)[0]
        es, ed = gsrc[em], dst[em] - nr * c
        blk = ed // P
        half = (es % 2).astype(np.int64)          # parity groups (even/odd row)
        gidx = np.zeros((2, cfg.SLOTS_H), np.int64)      # pad: row-pair 0
        doff = np.full((2, cfg.SLOTS_H), 255, np.int64)
        etyp = np.zeros((2, cfg.SLOTS_H), np.int64)
        for b in range(cfg.NBLK):
            for h in (0, 1):
                m = (blk == b) & (half == h)
                k = int(m.sum())
                assert k <= cfg.CAP_H, f"core {c} blk {b} par {h}: {k}>{cfg.CAP_H}"
                sl = slice(b * cfg.CAP_H, b * cfg.CAP_H + k)
                gidx[h, sl] = es[m] // 2
                doff[h, sl] = ed[m] - b * P
                etyp[h, sl] = etype_all[em][m]
        gb = batch[c * nr:(c + 1) * nr]
        # bond one-hot feed: [24, 2*SLOTS_H] fp8; rows 8f..8f+8 hold field f
        ets = np.concatenate([etyp[0], etyp[1]])
        fields = np.stack([ets // 64, (ets // 8) % 8, ets % 8])   # [3, S]
        ety24 = np.repeat(fields, 8, axis=0).astype(NP_FP8)       # [24, S]
        cores.append(dict(
            idxA=wrap16(gidx[0]), idxB=wrap16(gidx[1]),
            ety24=ety24,
            doff=np.concatenate(
                [doff[0].reshape(cfg.NTILE_H, P).T,
                 doff[1].reshape(cfg.NTILE_H, P).T], axis=1).astype(np.float32),
            batch_loc=gb))
    return cores


def build_shared_inputs(cfg, inputs):
    w = {}
    atom_emb = np.asarray(inputs["atom_emb"], np.float32)
    nv = atom_emb.shape[1]           # atom vocab (64)
    nf = atom_emb.shape[0]           # 9
    kch = -(-nf * nv // P)           # one-hot K chunks (5)
    ae_pad = np.zeros((kch * P, D), np.float32)
    ae_pad[:nf * nv] = atom_emb.reshape(nf * nv, D)
    # [P(k-row), kch, D] so partition dim is the contraction row
    w["atom_tab"] = np.ascontiguousarray(
        ae_pad.reshape(kch, P, D).transpose(1, 0, 2)).astype(NP_BF16)
    bond_emb = np.asarray(inputs["bond_emb"], np.float32)
    w["bond_tab"] = bond_emb.reshape(24, D).astype(NP_BF16)
    w["viter"] = np.tile(np.arange(8, dtype=np.float32), 3).reshape(24, 1)
    w["gcn_W"] = np.ascontiguousarray(
        np.asarray(inputs["gcn_W"], np.float32).transpose(1, 0, 2)
        .reshape(D, L * D)).astype(NP_FP16)
    w["vn_W1"] = np.ascontiguousarray(
        np.asarray(inputs["vn_W1"], np.float32).transpose(1, 0, 2)
        .reshape(D, (L - 1) * D))
    w["vn_W2"] = np.ascontiguousarray(
        np.asarray(inputs["vn_W2"], np.float32).transpose(1, 0, 2)
        .reshape(D, (L - 1) * D))
    iota = np.broadcast_to(np.arange(P, dtype=np.float32), (P, P))
    w["iota_row"] = np.ascontiguousarray(iota).astype(NP_BF16)
    ln = np.asarray(inputs["norm_g"], np.float32).reshape(-1)
    w["grep"] = np.broadcast_to(ln, (P, L * D)).copy()
    lb = np.asarray(inputs["norm_b"], np.float32).reshape(-1)
    w["brep"] = np.broadcast_to(lb, (P, L * D)).copy()
    gb = np.asarray(inputs["gcn_b"], np.float32).reshape(-1)
    w["gbrep"] = np.broadcast_to(gb, (P, L * D)).copy()
    vnr = np.asarray(inputs["vn_emb"], np.float32).reshape(-1)
    w["vnrep"] = np.broadcast_to(vnr, (P, D)).copy()
    return w


def build_core_inputs(cfg, core, shared, inputs):
    m = dict(shared)
    m.update({k: core[k] for k in ("idxA", "idxB", "ety24", "doff")})
    nr = cfg.NLOC_REAL
    bb = np.full((cfg.NLOC,), -1.0, np.float32)
    bb[:nr] = core["batch_loc"].astype(np.float32)
    bcols = bb.reshape(cfg.NBLK, P).T
    m["batch0"] = bcols.astype(np.float32)
    m["batch1"] = (bcols - 128.0).astype(np.float32)
    oh = np.zeros((cfg.GT, cfg.NBLK, P, P), NP_FP8)
    bi = bb.astype(np.int64)
    for t in range(cfg.NBLK):
        for p in range(P):
            g = bi[t * P + p]
            if g >= 0:
                oh[g // P, t, g % P, p] = 1.0
    m["oh_bat"] = oh
    # atom one-hots: [kch, NBLK, P(k), P(n)] for this core's nodes
    x = np.asarray(inputs["x"], np.int64)
    nf = x.shape[1]
    nv = np.asarray(inputs["atom_emb"]).shape[1]
    kch = -(-nf * nv // P)
    xs = np.zeros((cfg.NLOC, nf), np.int64)
    cid = int(core["cid"])
    xs[:nr] = x[cid * nr:(cid + 1) * nr]
    kv = (np.arange(nf) * nv)[None, :] + xs          # [NLOC, nf] in [0, nf*nv)
    ohx = np.zeros((kch, cfg.NBLK, P, P), NP_FP8)
    tt = np.arange(cfg.NLOC) // P
    pp = np.arange(cfg.NLOC) % P
    for f in range(nf):
        k = kv[:, f]
        valid = np.zeros(cfg.NLOC, bool)
        valid[:nr] = True
        ohx[k[valid] // P, tt[valid], k[valid] % P, pp[valid]] = 1.0
    m["atom_oh"] = np.ascontiguousarray(ohx.transpose(1, 2, 0, 3))  # [NBLK,P(k),kch,P(n)]
    return m


# ---------------- the SPMD bass program ----------------

def build_bass(cfg, w):
    nc = bacc.Bacc("TRN2", target_bir_lowering=False, debug=False,
                   num_devices=NC_)
    NBLK, TBH = cfg.NBLK, cfg.TBLK_H
    NTILE, NTILE_H = cfg.NTILE, cfg.NTILE_H
    NLOC, NPAD, HALF, GT = cfg.NLOC, cfg.NPAD, cfg.HALF, cfg.GT
    KCH = w["atom_tab"].shape[1]

    ln_triv = np.allclose(w["grep"], 1) and np.allclose(w["brep"], 0)
    gb_triv = np.allclose(w["gbrep"], 0)
    vn_triv = np.allclose(w["vnrep"], 0)

    def din(name, arr_shape, dt):
        return nc.dram_tensor(name, list(arr_shape), dt, kind="ExternalInput")

    atom_oh = din("atom_oh", [NBLK, P, KCH, P], FP8)
    atom_tab = din("atom_tab", [P, KCH, D], BF16)
    bond_tab = din("bond_tab", [24, D], BF16)
    viter_d = din("viter", [24, 1], F32)
    ety24_d = din("ety24", [24, 2 * cfg.SLOTS_H], FP8)
    idxA_d = din("idxA", [P, cfg.SLOTS_H // 16], I16)
    idxB_d = din("idxB", [P, cfg.SLOTS_H // 16], I16)
    doff_d = din("doff", [P, NTILE], F32)
    batch0_d = din("batch0", [P, NBLK], F32)
    batch1_d = din("batch1", [P, NBLK], F32)
    oh_bat_d = din("oh_bat", [GT, NBLK, P, P], FP8)
    gcnW_d = din("gcn_W", [D, L * D], FP16)
    vnW1_d = din("vn_W1", [D, (L - 1) * D], F32)
    vnW2_d = din("vn_W2", [D, (L - 1) * D], F32)
    iota_d = din("iota_row", [P, P], BF16)
    grep_d = din("grep", [P, L * D], F32)
    brep_d = din("brep", [P, L * D], F32)
    gbrep_d = din("gbrep", [P, L * D], F32)
    vnrep_d = din("vnrep", [P, D], F32)
    out_p = nc.dram_tensor("out", [NLOC, D], F32, kind="ExternalOutput")

    # h feed table: [NPAD, D] fp16 (+2 pad rows for the odd-pair view);
    # AllGather writes it directly (Shared scratchpad).
    hfull = nc.dram_tensor("hfull", [NPAD + 2, D], FP16, addr_space="Shared")
    shard_b = nc.dram_tensor("shard_b", [NLOC, D], FP16)
    vt_in = nc.dram_tensor("vt_in", [GT * P, D], F32)
    vt_out = nc.dram_tensor("vt_out", [GT * P, D], F32)
    RG = [list(range(NC_))]
    A = mybir.AluOpType
    AF = mybir.ActivationFunctionType

    with tile.TileContext(nc) as tc:
        with tc.tile_pool(name="res", bufs=1) as res, \
             tc.tile_pool(name="wk", bufs=2) as wk, \
             tc.tile_pool(name="gat", bufs=3) as gat, \
             tc.tile_pool(name="psA", bufs=2, space="PSUM") as psA, \
             tc.tile_pool(name="psV", bufs=1, space="PSUM") as psV, \
             tc.tile_pool(name="psT", bufs=1, space="PSUM") as psT, \
             tc.tile_pool(name="psM", bufs=2, space="PSUM") as psM:

            ST = res.tile([P, NTILE, P], FP8, tag="ST")
            EM = res.tile([P, NTILE, D], FP16 if EM_DT_ENV else FP8, tag="EM")
            IOTA = res.tile([P, P], BF16, tag="IOTA")
            IDENT = res.tile([P, P], F32, tag="IDENT")
            HRES = res.tile([P, NBLK, D], F32, tag="HRES")
            HFEED = res.tile([P, NBLK, D], FP16, tag="HFEED")
            H2 = res.tile([P, NBLK, D], FP16, tag="H2")
            OHT = res.tile([P, GT * NBLK, P], FP8, tag="OHT")
            OHBT = res.tile([P, GT * NBLK, P], FP8, tag="OHBT")
            VNT = res.tile([P, GT, D], F32, tag="VNT")
            VNT16 = res.tile([P, GT, D], FP16, tag="VNT16")
            GW = res.tile([D, L * D], FP16, tag="GW")
            VW1 = res.tile([D, (L - 1) * D], F32, tag="VW1")
            VW2 = res.tile([D, (L - 1) * D], F32, tag="VW2")
            MU = res.tile([P, NBLK], F32, tag="MU")
            SSQ = res.tile([P, NBLK], F32, tag="SSQ")
            RS = res.tile([P, NBLK], F32, tag="RS")
            GREP = BREP = GBREP = VNREP = None
            if not ln_triv:
                GREP = res.tile([P, L * D], F32, tag="GREP")
                BREP = res.tile([P, L * D], F32, tag="BREP")
            if not gb_triv:
                GBREP = res.tile([P, L * D], F32, tag="GBREP")
            if not vn_triv:
                VNREP = res.tile([P, D], F32, tag="VNREP")

            nc.sync.dma_start(out=IOTA[:], in_=iota_d[:])
            make_identity(nc, IDENT[:])
            nc.sync.dma_start(out=GW[:], in_=gcnW_d[:])
            nc.sync.dma_start(out=VW1[:], in_=vnW1_d[:])
            nc.sync.dma_start(out=VW2[:], in_=vnW2_d[:])
            nc.sync.dma_start(
                out=OHBT[:], in_=oh_bat_d[:].rearrange("q t p c -> p (q t) c"))
            if not ln_triv:
                nc.sync.dma_start(out=GREP[:], in_=grep_d[:])
                nc.sync.dma_start(out=BREP[:], in_=brep_d[:])
            if not gb_triv:
                nc.sync.dma_start(out=GBREP[:], in_=gbrep_d[:])
            if not vn_triv:
                nc.sync.dma_start(out=VNREP[:], in_=vnrep_d[:])

            # zero the 2 pad rows past the AllGather output (read by the
            # odd-pair gather view)
            ZT = res.tile([P, D], FP16, tag="ZT")
            nc.vector.memset(ZT[:], 0.0)
            nc.sync.dma_start(out=hfull[NPAD:NPAD + 2, :], in_=ZT[0:2, :])
            BIX = res.tile([P, 1], I16, tag="BIX")
            nc.vector.memset(BIX[:], 0)
            BBUF = res.tile([P, 1, 2 * D], FP16, tag="BBUF")

            # ----- one-hot builds + bond EM matmuls (upfront) -----
            with tc.tile_pool(name="su", bufs=2) as su:
                DOFF = su.tile([P, NTILE], F32, tag="DOFF", bufs=1)
                B0 = su.tile([P, NBLK], F32, tag="B0", bufs=1)
                B1 = su.tile([P, NBLK], F32, tag="B1", bufs=1)
                nc.sync.dma_start(out=DOFF[:], in_=doff_d[:])
                nc.sync.dma_start(out=B0[:], in_=batch0_d[:])
                nc.sync.dma_start(out=B1[:], in_=batch1_d[:])
                for t in range(NTILE):
                    nc.vector.tensor_scalar(out=ST[:, t, :], in0=IOTA[:],
                                            scalar1=DOFF[:, t:t + 1],
                                            scalar2=None, op0=A.is_equal)
                for q in range(GT):
                    bq = B0 if q == 0 else B1
                    for t in range(NBLK):
                        nc.vector.tensor_scalar(
                            out=OHT[:, q * NBLK + t, :], in0=IOTA[:],
                            scalar1=bq[:, t:t + 1], scalar2=None,
                            op0=A.is_equal)
                BT24 = su.tile([24, D], BF16, tag="bt24", bufs=1)
                nc.sync.dma_start(out=BT24[:], in_=bond_tab[:])
                VIT = su.tile([24, 1], F32, tag="vit", bufs=1)
                nc.sync.dma_start(out=VIT[:], in_=viter_d[:])
                ECH = 16
                for c0 in range(0, NTILE, ECH):
                    nch = min(ECH, NTILE - c0)
                    ET = su.tile([24, ECH * P], FP8, tag="et")
                    nc.sync.dma_start(out=ET[:, 0:nch * P],
                                      in_=ety24_d[:, c0 * P:(c0 + nch) * P])
                    for j in range(nch):
                        OH24 = su.tile([24, P], BF16, tag="oh24")
                        nc.vector.tensor_scalar(
                            out=OH24[:], in0=ET[:, j * P:(j + 1) * P],
                            scalar1=VIT[:], scalar2=None, op0=A.is_equal)
                        pem = psM.tile([P, D], F32, tag="pmm")
                        nc.tensor.matmul(out=pem[:], lhsT=OH24[:], rhs=BT24[:],
                                         start=True, stop=True)
                        nc.scalar.copy(out=EM[:, c0 + j, :], in_=pem[:])

            # ----- atom embeddings -> h0, feed0 -----
            with tc.tile_pool(name="at", bufs=2) as at:
                atab = at.tile([P, KCH, D], BF16, tag="atab")
                nc.sync.dma_start(out=atab[:], in_=atom_tab[:])
                for t in range(NBLK):
                    ohx = at.tile([P, KCH, P], FP8, tag="ohx")
                    nc.sync.dma_start(out=ohx[:], in_=atom_oh[t])
                    ph = psM.tile([P, D], F32, tag="pmm")
                    for k in range(KCH):
                        nc.tensor.matmul(out=ph[:], lhsT=ohx[:, k, :],
                                         rhs=atab[:, k, :],
                                         start=(k == 0), stop=(k == KCH - 1))
                    if vn_triv:
                        nc.vector.tensor_copy(out=HRES[:, t, :], in_=ph[:])
                    else:
                        nc.vector.tensor_tensor(out=HRES[:, t, :], in0=ph[:],
                                                in1=VNREP[:], op=A.add)
                    nc.vector.tensor_copy(out=HFEED[:, t, :], in_=HRES[:, t, :])

            # ----- helpers -----
            def ln_relu(dst, src_ap, li, relu):
                mu = wk.tile([P, 1], F32, tag="mu")
                nc.vector.tensor_reduce(out=mu[:], in_=src_ap, op=A.add,
                                        axis=mybir.AxisListType.X)
                nc.vector.tensor_scalar(out=mu[:], in0=mu[:], scalar1=1.0 / D,
                                        scalar2=None, op0=A.mult)
                dt_ = wk.tile([P, D], F32, tag="lnd")
                nc.vector.tensor_scalar(out=dt_[:], in0=src_ap, scalar1=mu[:],
                                        scalar2=None, op0=A.subtract)
                jk = wk.tile([P, D], F32, tag="lnj")
                ssq = wk.tile([P, 1], F32, tag="ssq")
                nc.scalar.activation(out=jk[:], in_=dt_[:], func=AF.Square,
                                     accum_out=ssq[:])
                nc.vector.tensor_scalar(out=ssq[:], in0=ssq[:], scalar1=1.0 / D,
                                        scalar2=LN_EPS, op0=A.mult, op1=A.add)
                nc.scalar.sqrt(out=ssq[:], in_=ssq[:])
                rs = wk.tile([P, 1], F32, tag="rs")
                nc.vector.reciprocal(out=rs[:], in_=ssq[:])
                if ln_triv or li is None:
                    if relu:
                        nc.vector.tensor_scalar(
                            out=dst, in0=dt_[:], scalar1=rs[:], scalar2=0.0,
                            op0=A.mult, op1=A.max)
                    else:
                        nc.vector.tensor_scalar(
                            out=dst, in0=dt_[:], scalar1=rs[:], scalar2=None,
                            op0=A.mult)
                else:
                    t1 = wk.tile([P, D], F32, tag="lnt1")
                    nc.vector.tensor_scalar(out=t1[:], in0=dt_[:], scalar1=rs[:],
                                            scalar2=None, op0=A.mult)
                    t2 = wk.tile([P, D], F32, tag="lnt2")
                    nc.vector.tensor_tensor(out=t2[:], in0=t1[:],
                                            in1=GREP[:, li * D:(li + 1) * D],
                                            op=A.mult)
                    t3 = wk.tile([P, D], F32, tag="lnt3")
                    nc.vector.tensor_tensor(out=t3[:], in0=t2[:],
                                            in1=BREP[:, li * D:(li + 1) * D],
                                            op=A.add)
                    if relu:
                        nc.vector.tensor_scalar(out=dst, in0=t3[:], scalar1=0.0,
                                                scalar2=None, op0=A.max)
                    else:
                        nc.vector.tensor_copy(out=dst, in_=t3[:])

            def write_shard_allgather(prev_gathers):
                sh_bi = nc.sync.dma_start(
                    out=shard_b[:].rearrange("(a p) d -> p a d", p=P),
                    in_=HFEED[:])
                ag = nc.gpsimd.collective_compute(
                    "AllGather", A.bypass, replica_groups=RG,
                    ins=[shard_b[:]], outs=[hfull[0:NPAD, :]])
                for gprev in prev_gathers:
                    add_dep_helper(ag.ins, gprev.ins, reason="AG after gathers")
                return ag

            # pair views over hfull for the parity gathers: stride 2 rows
            # (256B), element = 2 rows (256B); wanted row is the first half.
            _hap = hfull[:]
            evenv = dataclasses.replace(
                _hap, ap=type(_hap.ap)([[2 * D, HALF], [1, 2 * D]]))
            oddv = dataclasses.replace(evenv, offset=D)
            dma_sem = nc.alloc_semaphore("swdge_pref")
            PF = min(int(os.environ.get("K_PF", "0")), cfg.NCHUNK)
            sem_cum = [0]
            last_wt = [None]

            def prep_chunk(ch):
                """prepare_only gathers for chunk ch (descriptors written now,
                DMA fired by the next trigger_dma; hfull dep defers there)."""
                bufA = gat.tile([P, cfg.CH_TILES, 2 * D], FP16, tag="gA")
                bufB = gat.tile([P, cfg.CH_TILES, 2 * D], FP16, tag="gB")
                c0 = ch * cfg.CH_IDX // 16
                ixA = gat.tile([P, cfg.CH_IDX // 16], I16, tag="ixA")
                ixB = gat.tile([P, cfg.CH_IDX // 16], I16, tag="ixB")
                dA = nc.sync.dma_start(out=ixA[:],
                                       in_=idxA_d[:, c0:c0 + cfg.CH_IDX // 16])
                dB = nc.sync.dma_start(out=ixB[:],
                                       in_=idxB_d[:, c0:c0 + cfg.CH_IDX // 16])
                if last_wt[0] is not None:
                    add_dep_helper(dA.ins, last_wt[0].ins,
                                   reason="ix reuse after drains")
                    add_dep_helper(dB.ins, last_wt[0].ins,
                                   reason="ix reuse after drains")
                pA = nc.gpsimd.dma_gather(
                    out_ap=bufA[:], in_ap=evenv, idxs_ap=ixA[:],
                    num_idxs=cfg.CH_IDX, num_idxs_reg=cfg.CH_IDX,
                    elem_size=2 * D, single_packet=False,
                    prepare_only=True, sem=dma_sem)
                pB = nc.gpsimd.dma_gather(
                    out_ap=bufB[:], in_ap=oddv, idxs_ap=ixB[:],
                    num_idxs=cfg.CH_IDX, num_idxs_reg=cfg.CH_IDX,
                    elem_size=2 * D, single_packet=False,
                    prepare_only=True, sem=dma_sem)
                return (bufA, bufB, pA, pB)

            def edge_phase(l, ag_bi, fuse_stats, prefetched=()):
                gathers = []
                Wl = GW[:, l * D:(l + 1) * D]
                trigger = None
                wt = None
                if prefetched:
                    # gate the trigger behind the AG through a Pool engine
                    # nop: deps on the nop materialize as a Collectives-sem
                    # wait, and the trigger's engine-lane wait covers the nop
                    # (a direct collective dep on InstTriggerDma is dropped).
                    xnop = nc.gpsimd.engine_nop()
                    if ag_bi is not None:
                        add_dep_helper(xnop.ins, ag_bi.ins, reason="AG gate")
                    trigger = nc.gpsimd.trigger_dma(count=None)
                    add_dep_helper(trigger.ins, xnop.ins,
                                   reason="trigger after AG gate")
                    # sim-contract wait: each prep incs the sem by 16 at
                    # drain (HW granularity differs; the barrier gather below
                    # provides the real ring-FIFO ordering there).
                    sem_cum[0] += 16 * 2 * len(prefetched)
                    wts = nc.gpsimd.wait_ge(dma_sem, sem_cum[0])
                    add_dep_helper(wts.ins, trigger.ins,
                                   reason="sem wait after trigger")
                    # barrier gather: the SWDGE ring drains FIFO per engine,
                    # so this gather's completion implies every triggered
                    # prefetch drain has landed.
                    wt = nc.gpsimd.dma_gather(
                        out_ap=BBUF[:], in_ap=evenv, idxs_ap=BIX[:],
                        num_idxs=16, num_idxs_reg=16, elem_size=2 * D,
                        single_packet=False)
                    add_dep_helper(wt.ins, wts.ins,
                                   reason="ring barrier after sem wait")
                    last_wt[0] = wt
                    gathers.append(trigger)
                    gathers.append(wt)
                    for (_, _, pA, pB) in prefetched:
                        gathers += [pA, pB]
                for ch in range(cfg.NCHUNK):
                    if ch < len(prefetched):
                        bufA, bufB = prefetched[ch][0], prefetched[ch][1]
                    else:
                        bufA = gat.tile([P, cfg.CH_TILES, 2 * D], FP16,
                                        tag="gA")
                        bufB = gat.tile([P, cfg.CH_TILES, 2 * D], FP16,
                                        tag="gB")
                        c0 = ch * cfg.CH_IDX // 16
                        ixA = gat.tile([P, cfg.CH_IDX // 16], I16, tag="ixA")
                        ixB = gat.tile([P, cfg.CH_IDX // 16], I16, tag="ixB")
                        nc.sync.dma_start(
                            out=ixA[:], in_=idxA_d[:, c0:c0 + cfg.CH_IDX // 16])
                        nc.sync.dma_start(
                            out=ixB[:], in_=idxB_d[:, c0:c0 + cfg.CH_IDX // 16])
                        gA = nc.gpsimd.dma_gather(
                            out_ap=bufA[:], in_ap=evenv, idxs_ap=ixA[:],
                            num_idxs=cfg.CH_IDX, num_idxs_reg=cfg.CH_IDX,
                            elem_size=2 * D, single_packet=False)
                        gB = nc.gpsimd.dma_gather(
                            out_ap=bufB[:], in_ap=oddv, idxs_ap=ixB[:],
                            num_idxs=cfg.CH_IDX, num_idxs_reg=cfg.CH_IDX,
                            elem_size=2 * D, single_packet=False)
                        if ag_bi is not None:
                            add_dep_helper(gA.ins, ag_bi.ins,
                                           reason="gather after AG")
                            add_dep_helper(gB.ins, ag_bi.ins,
                                           reason="gather after AG")
                        if trigger is not None:
                            add_dep_helper(gA.ins, trigger.ins,
                                           reason="ring order after trigger")
                            add_dep_helper(gB.ins, trigger.ins,
                                           reason="ring order after trigger")
                        gathers += [gA, gB]
                    rhs2 = []
                    for half, buf in ((0, bufA), (1, bufB)):
                        # tt lives in the gathered buffer's spare half
                        tt = buf[:, :, D:2 * D]
                        rhs = wk.tile([P, cfg.CH_TILES, 2 * D], FP16,
                                      tag=f"rhs{half}")
                        uu = rhs[:, :, 0:D]
                        em_sl = EM[:, half * NTILE_H + ch * cfg.CH_TILES:
                                   half * NTILE_H + (ch + 1) * cfg.CH_TILES, :]
                        ttadd = nc.vector.tensor_tensor(
                            out=tt, in0=buf[:, :, 0:D], in1=em_sl, op=A.add)
                        if ch < len(prefetched) and wt is not None:
                            add_dep_helper(ttadd.ins, wt.ins,
                                           reason="read after prefetch drain")
                        nc.scalar.activation(out=uu, in_=tt, func=AF.Exp)
                        nc.vector.scalar_tensor_tensor(
                            out=rhs[:, :, D:2 * D], in0=tt, scalar=0.0,
                            in1=uu, op0=A.max, op1=A.mult)
                        nc.vector.tensor_scalar(out=uu, in0=uu,
                                                scalar1=1.0, scalar2=None,
                                                op0=A.max)
                        rhs2.append(rhs)
                    for bb in range(cfg.CHUNK_BLKS):
                        b = ch * cfg.CHUNK_BLKS + bb
                        pb = psA.tile([P, 2 * D], F32, tag="blk")
                        for half in (0, 1):
                            for j in range(TBH):
                                gt_id = half * NTILE_H + b * TBH + j
                                nc.tensor.matmul(
                                    out=pb[:], lhsT=ST[:, gt_id, :],
                                    rhs=rhs2[half][:, bb * TBH + j, :],
                                    start=(half == 0 and j == 0),
                                    stop=(half == 1 and j == TBH - 1))
                        dmx = wk.tile([P, D], F32, tag="dmx")
                        nc.vector.tensor_scalar(out=dmx[:], in0=pb[:, 0:D],
                                                scalar1=1e-16, scalar2=None,
                                                op0=A.max)
                        rcp = wk.tile([P, D], F32, tag="rcp")
                        nc.vector.reciprocal(out=rcp[:], in_=dmx[:])
                        mlpin = wk.tile([P, D], F32, tag="mlpin")
                        nc.vector.tensor_tensor(out=mlpin[:], in0=pb[:, D:2 * D],
                                                in1=rcp[:], op=A.mult)
                        nc.vector.scalar_tensor_tensor(
                            out=mlpin[:], in0=mlpin[:], scalar=MSG_EPS,
                            in1=HFEED[:, b, :], op0=A.add, op1=A.add)
                        pxt = psT.tile([D, P], F32, tag="pxt")
                        nc.tensor.transpose(out=pxt[:], in_=mlpin[:],
                                            identity=IDENT[:])
                        xt = wk.tile([D, P], FP16, tag="xt")
                        nc.scalar.copy(out=xt[:], in_=pxt[:])
                        ph2 = psM.tile([P, D], F32, tag="pmm")
                        nc.tensor.matmul(out=ph2[:], lhsT=xt[:], rhs=Wl,
                                         start=True, stop=True)
                        if l == 0 and gb_triv:
                            nc.vector.tensor_copy(out=HRES[:, b, :], in_=ph2[:])
                        elif l == 0:
                            nc.vector.tensor_tensor(
                                out=HRES[:, b, :], in0=ph2[:],
                                in1=GBREP[:, l * D:(l + 1) * D], op=A.add)
                        else:
                            nc.vector.tensor_tensor(out=HRES[:, b, :],
                                                    in0=ph2[:],
                                                    in1=HRES[:, b, :], op=A.add)
                            if not gb_triv:
                                nc.vector.tensor_tensor(
                                    out=HRES[:, b, :], in0=HRES[:, b, :],
                                    in1=GBREP[:, l * D:(l + 1) * D], op=A.add)
                        if fuse_stats:
                            # LN row stats of HRES[b] (sum and sum-of-squares)
                            # computed inline; sqrt/reciprocal batched later.
                            nc.vector.tensor_reduce(
                                out=MU[:, b:b + 1], in_=HRES[:, b, :],
                                op=A.add, axis=mybir.AxisListType.X)
                            sqt = wk.tile([P, D], F32, tag="sqt")
                            nc.vector.tensor_tensor(out=sqt[:],
                                                    in0=HRES[:, b, :],
                                                    in1=HRES[:, b, :],
                                                    op=A.mult)
                            nc.vector.tensor_reduce(
                                out=SSQ[:, b:b + 1], in_=sqt[:],
                                op=A.add, axis=mybir.AxisListType.X)
                return gathers

            def batch_ln_stats():
                # MU <- mean, RS <- 1/sqrt(var+eps), one [P, NBLK] pass
                nc.vector.tensor_scalar(out=MU[:], in0=MU[:], scalar1=1.0 / D,
                                        scalar2=None, op0=A.mult)
                msq = wk.tile([P, NBLK], F32, tag="bmsq")
                nc.vector.tensor_tensor(out=msq[:], in0=MU[:], in1=MU[:],
                                        op=A.mult)
                var = wk.tile([P, NBLK], F32, tag="bvar")
                nc.vector.tensor_scalar(out=var[:], in0=SSQ[:],
                                        scalar1=1.0 / D, scalar2=None,
                                        op0=A.mult)
                nc.vector.tensor_tensor(out=var[:], in0=var[:], in1=msq[:],
                                        op=A.subtract)
                # clamp before +eps: one-pass var can go slightly negative
                nc.vector.tensor_scalar(out=var[:], in0=var[:], scalar1=0.0,
                                        scalar2=LN_EPS, op0=A.max, op1=A.add)
                sq = wk.tile([P, NBLK], F32, tag="bsq")
                nc.scalar.sqrt(out=sq[:], in_=var[:])
                nc.vector.reciprocal(out=RS[:], in_=sq[:])

            # ===== layer 0 =====
            pref = [prep_chunk(ch) for ch in range(PF)]
            ag = write_shard_allgather([])
            gathers = edge_phase(0, ag, fuse_stats=ln_triv, prefetched=pref)

            # ===== layers 1..L-1 =====
            for l in range(1, L):
                pvt = []
                for q in range(GT):
                    pvtq = psV.tile([P, D], F32, tag=f"vt{q}", name=f"pvt{q}")
                    pvt.append(pvtq)
                if ln_triv:
                    batch_ln_stats()
                for t in range(NBLK):
                    if ln_triv:
                        h2t = wk.tile([P, D], F32, tag="h2t")
                        nc.vector.tensor_scalar(
                            out=h2t[:], in0=HRES[:, t, :],
                            scalar1=MU[:, t:t + 1], scalar2=RS[:, t:t + 1],
                            op0=A.subtract, op1=A.mult)
                        nc.vector.tensor_scalar(
                            out=H2[:, t, :], in0=h2t[:], scalar1=0.0,
                            scalar2=None, op0=A.max)
                    else:
                        ln_relu(H2[:, t, :], HRES[:, t, :], l - 1, True)
                    for q in range(GT):
                        nc.tensor.matmul(out=pvt[q][:],
                                         lhsT=OHT[:, q * NBLK + t, :],
                                         rhs=H2[:, t, :], start=(t == 0),
                                         stop=(t == NBLK - 1),
                                         skip_group_check=True)
                vtl = wk.tile([P, GT, D], F32, tag="vtl")
                for q in range(GT):
                    if l == 1 and vn_triv:
                        nc.vector.tensor_copy(out=vtl[:, q, :], in_=pvt[q][:])
                    elif l == 1:
                        nc.vector.tensor_tensor(out=vtl[:, q, :], in0=pvt[q][:],
                                                in1=VNREP[:], op=A.add)
                    else:
                        nc.vector.tensor_tensor(out=vtl[:, q, :], in0=pvt[q][:],
                                                in1=VNT[:, q, :], op=A.add)
                nc.sync.dma_start(
                    out=vt_in[:].rearrange("(a p) d -> p a d", p=P), in_=vtl[:])
                ar = nc.gpsimd.collective_compute(
                    "AllReduce", A.add, replica_groups=RG,
                    ins=[vt_in[:]], outs=[vt_out[:]])
                vtr = wk.tile([P, GT, D], F32, tag="vtr")
                r_bi = nc.sync.dma_start(
                    out=vtr[:], in_=vt_out[:].rearrange("(a p) d -> p a d", p=P))
                add_dep_helper(r_bi.ins, ar.ins, reason="read after AR")
                # prep next layer's first chunks while Pool idles through
                # the AR/vn-MLP/AllGather window
                pref = [prep_chunk(ch) for ch in range(PF)]

                def vn_mlp(src_t, Wsl, dst_f32, dst_f16):
                    for q in range(GT):
                        pxt = psT.tile([D, P], F32, tag="pxt")
                        nc.tensor.transpose(out=pxt[:], in_=src_t[:, q, :],
                                            identity=IDENT[:])
                        xt = wk.tile([D, P], F32, tag="xtf")
                        nc.scalar.copy(out=xt[:], in_=pxt[:])
                        pu = psM.tile([P, D], F32, tag="pmm")
                        nc.tensor.matmul(out=pu[:], lhsT=xt[:], rhs=Wsl,
                                         start=True, stop=True)
                        uf = wk.tile([P, D], F32, tag="uf")
                        nc.vector.tensor_copy(out=uf[:], in_=pu[:])
                        ln_relu(dst_f32[:, q, :], uf[:], None, True)
                        if dst_f16 is not None:
                            nc.vector.tensor_copy(out=dst_f16[:, q, :],
                                                  in_=dst_f32[:, q, :])

                u1 = wk.tile([P, GT, D], F32, tag="u1")
                vn_mlp(vtr, VW1[:, (l - 1) * D:l * D], u1, None)
                vn_mlp(u1, VW2[:, (l - 1) * D:l * D], VNT, VNT16)

                for t in range(NBLK):
                    pv = psM.tile([P, D], F32, tag="pmm")
                    for q in range(GT):
                        nc.tensor.matmul(out=pv[:],
                                         lhsT=OHBT[:, q * NBLK + t, :],
                                         rhs=VNT16[:, q, :], start=(q == 0),
                                         stop=(q == GT - 1))
                    nc.vector.tensor_tensor(out=HFEED[:, t, :],
                                            in0=H2[:, t, :], in1=pv[:],
                                            op=A.add)
                ag = write_shard_allgather(gathers)
                gathers = edge_phase(l, ag, fuse_stats=ln_triv,
                                     prefetched=pref)

            # ===== output layernorm =====
            if ln_triv:
                batch_ln_stats()
                for t in range(NBLK):
                    ot = wk.tile([P, D], F32, tag="ot")
                    nc.vector.tensor_scalar(
                        out=ot[:], in0=HRES[:, t, :], scalar1=MU[:, t:t + 1],
                        scalar2=RS[:, t:t + 1], op0=A.subtract, op1=A.mult)
                    nc.sync.dma_start(out=out_p[t * P:(t + 1) * P, :],
                                      in_=ot[:])
            else:
                for t in range(NBLK):
                    ot = wk.tile([P, D], F32, tag="ot")
                    ln_relu(ot[:], HRES[:, t, :], L - 1, False)
                    nc.sync.dma_start(out=out_p[t * P:(t + 1) * P, :],
                                      in_=ot[:])

    nc.compile()
    return nc


# ---------------- driver ----------------

_CACHE = {}


def run_cfg(cfg, inputs, trace=False):
    key = (cfg.N, cfg.E, cfg.G, cfg.NBLK, cfg.TBLK_H)
    cores = build_layout(cfg, inputs["edge_index"], inputs["edge_attr"],
                         inputs["batch"])
    for c in range(NC_):
        cores[c]["cid"] = c
    shared = build_shared_inputs(cfg, inputs)
    if key not in _CACHE:
        _CACHE[key] = build_bass(cfg, shared)
    nc = _CACHE[key]
    in_maps = [build_core_inputs(cfg, cores[c], shared, inputs)
               for c in range(NC_)]
    if os.environ.get("K_SIM", "0") == "1":
        from concourse.bass_interp import MultiCoreSim
        sim = MultiCoreSim(nc, num_cores=NC_, require_finite=False,
                           require_nnan=False)
        for c, cs in enumerate(sim.cores.values()):
            for k, v in in_maps[c].items():
                cs.tensor(k)[:] = v
        sim.simulate(check_with_hw=False)
        nr = cfg.NLOC_REAL
        outp = np.zeros((cfg.N, D), np.float32)
        for c, cs in enumerate(sim.cores.values()):
            outp[c * nr:(c + 1) * nr] = np.asarray(cs.tensor("out"))[:nr]
        return outp, None
    import importlib.util as _ilu
    hook_py = "/opt/trn_rl_repo/antenv/axon_hooks.py"
    if trace and os.path.exists(hook_py) and "antenv.axon_hooks" not in sys.modules:
        try:
            _spec = _ilu.spec_from_file_location("antenv.axon_hooks", hook_py)
            _mod = _ilu.module_from_spec(_spec)
            _spec.loader.exec_module(_mod)
            sys.modules["antenv.axon_hooks"] = _mod
        except Exception:
            trace = False
    from concourse.bass_utils import run_bass_kernel_spmd
    res = run_bass_kernel_spmd(nc, in_maps, list(range(NC_)), trace=trace)
    nr = cfg.NLOC_REAL
    outp = np.zeros((cfg.N, D), np.float32)
    for c in range(NC_):
        outp[c * nr:(c + 1) * nr] = res.results[c]["out"][:nr]
    return outp, res


def kernel(**inputs):
    cfg = CFG.full()
    out, _ = run_cfg(cfg, inputs, trace=False)
    return out



# revision 38
# speedup vs baseline: 1.0336x; 1.0336x over previous
"""DeeperGCN (GENConv softmax-aggr + virtual node) on 8 Trainium2 NeuronCores.

Self-contained kernel: host-side index preprocessing (graph partitioning /
slot layout only), one static SPMD Bass/Tile program compiled for 8 cores,
executed via concourse.bass_utils.run_bass_kernel_spmd.

Distribution (static program, no data-dependent control flow):
  - nodes sharded contiguously; core c owns original nodes [6250c, 6250(c+1)),
    padded to NLOC=6400 (NBLK=50 blocks x 128).
  - edges partitioned by dst owner, grouped per (block, src-table-half) into
    fixed tile slots; slot -> (tile, partition) = (slot//128, slot%128).
  - per layer: feed vectors AllGathered into an fp16 HBM table [NPAD, 64];
    per-edge source rows fetched with dma_gather (256B descriptors via the
    elem_step=64 two-row trick; int16 indices fit because each table half has
    NPAD/2 = 25600 rows); messages exp/weighted on ACT/DVE; per-dst-block
    softmax sums via one-hot matmuls (S^T fp8, device-built once) into PSUM.
  - virtual-node pooled embeddings [G, 64] AllReduced per layer; vn[batch]
    re-expansion also via one-hot matmuls.
"""
import sys

sys.path.insert(0, "/opt/trn_rl_repo")

import os
import numpy as np
import ml_dtypes
EM_DT_ENV = os.environ.get("K_EM16", "0") == "1"

import concourse.bass as bass
import concourse.bacc as bacc
import concourse.tile as tile
import concourse.mybir as mybir
from concourse.tile_rust import add_dep_helper
from concourse.masks import make_identity
import dataclasses


def _two_row_view(ap, rows, two_d):
    """Overlapping-row AP view: row stride D, row length 2D (gather trick)."""
    return dataclasses.replace(ap, ap=type(ap.ap)([[two_d // 2, rows],
                                                   [1, two_d]]))

# problem constants
N, E, D, G_FULL, L = 50000, 400000, 64, 256, 4
MSG_EPS = 1e-7
LN_EPS = 1e-5
NC_ = 8
P = 128

FP16, FP8, BF16, F32, I16 = (mybir.dt.float16, mybir.dt.float8e4,
                             mybir.dt.bfloat16, mybir.dt.float32,
                             mybir.dt.int16)
NP_FP16, NP_FP8, NP_BF16 = np.float16, ml_dtypes.float8_e4m3, ml_dtypes.bfloat16


class CFG:
    def __init__(self, n, e, g, nblk, tblk_h, chunk_blks):
        self.N, self.E, self.G = n, e, g
        self.NBLK, self.TBLK_H = nblk, tblk_h
        self.CAP_H = tblk_h * P
        self.NLOC = nblk * P
        self.NPAD = self.NLOC * NC_
        self.HALF = self.NPAD // 2
        self.NLOC_REAL = n // NC_
        self.SLOTS_H = nblk * self.CAP_H
        self.NTILE_H = nblk * tblk_h
        self.NTILE = 2 * self.NTILE_H
        self.CHUNK_BLKS = chunk_blks
        assert nblk % chunk_blks == 0
        self.NCHUNK = nblk // chunk_blks
        self.CH_TILES = chunk_blks * tblk_h
        self.CH_IDX = self.CH_TILES * P
        self.GT = max(1, g // P)

    @staticmethod
    def full():
        return CFG(N, E, G_FULL, 50, 5, int(os.environ.get('K_CB', '2')))

    @staticmethod
    def small():
        return CFG(8192, 24576, 64, 8, 2, 4)


# ---------------- host-side layout (pure index work) ----------------

def build_layout(cfg, edge_index, edge_attr, batch):
    src = np.asarray(edge_index[0], np.int64)
    dst = np.asarray(edge_index[1], np.int64)
    batch = np.asarray(batch, np.int64)
    ea = np.asarray(edge_attr, np.int64)
    etype_all = ea[:, 0] * 64 + ea[:, 1] * 8 + ea[:, 2]

    nr, nl = cfg.NLOC_REAL, cfg.NLOC
    c_of = src // nr
    gsrc = nl * c_of + (src - nr * c_of)
    owner = dst // nr

    def wrap16(lin):
        w = np.zeros((P, len(lin) // 16), np.int16)
        cols = np.arange(len(lin)) // 16
        rows = np.arange(len(lin)) % 16
        for r in range(8):
            w[rows + 16 * r, cols] = lin.astype(np.int16)
        return w

    cores = []
    for c in range(NC_):
        em = np.nonzero(owner == c)[0]
        es, ed = gsrc[em], dst[em] - nr * c
        blk = ed // P
        half = (es % 2).astype(np.int64)          # parity groups (even/odd row)
        gidx = np.zeros((2, cfg.SLOTS_H), np.int64)      # pad: row-pair 0
        doff = np.full((2, cfg.SLOTS_H), 255, np.int64)
        etyp = np.zeros((2, cfg.SLOTS_H), np.int64)
        for b in range(cfg.NBLK):
            for h in (0, 1):
                m = (blk == b) & (half == h)
                k = int(m.sum())
                assert k <= cfg.CAP_H, f"core {c} blk {b} par {h}: {k}>{cfg.CAP_H}"
                sl = slice(b * cfg.CAP_H, b * cfg.CAP_H + k)
                gidx[h, sl] = es[m] // 2
                doff[h, sl] = ed[m] - b * P
                etyp[h, sl] = etype_all[em][m]
        gb = batch[c * nr:(c + 1) * nr]
        # bond one-hot feed: [24, 2*SLOTS_H] fp8; rows 8f..8f+8 hold field f
        ets = np.concatenate([etyp[0], etyp[1]])
        fields = np.stack([ets // 64, (ets // 8) % 8, ets % 8])   # [3, S]
        ety24 = np.repeat(fields, 8, axis=0).astype(NP_FP8)       # [24, S]
        cores.append(dict(
            idxA=wrap16(gidx[0]), idxB=wrap16(gidx[1]),
            ety24=ety24,
            doff=np.concatenate(
                [doff[0].reshape(cfg.NTILE_H, P).T,
                 doff[1].reshape(cfg.NTILE_H, P).T], axis=1).astype(np.float32),
            batch_loc=gb))
    return cores


def build_shared_inputs(cfg, inputs):
    w = {}
    atom_emb = np.asarray(inputs["atom_emb"], np.float32)
    nv = atom_emb.shape[1]           # atom vocab (64)
    nf = atom_emb.shape[0]           # 9
    kch = -(-nf * nv // P)           # one-hot K chunks (5)
    ae_pad = np.zeros((kch * P, D), np.float32)
    ae_pad[:nf * nv] = atom_emb.reshape(nf * nv, D)
    # [P(k-row), kch, D] so partition dim is the contraction row
    w["atom_tab"] = np.ascontiguousarray(
        ae_pad.reshape(kch, P, D).transpose(1, 0, 2)).astype(NP_BF16)
    bond_emb = np.asarray(inputs["bond_emb"], np.float32)
    w["bond_tab"] = bond_emb.reshape(24, D).astype(NP_BF16)
    w["viter"] = np.tile(np.arange(8, dtype=np.float32), 3).reshape(24, 1)
    w["gcn_W"] = np.ascontiguousarray(
        np.asarray(inputs["gcn_W"], np.float32).transpose(1, 0, 2)
        .reshape(D, L * D)).astype(NP_FP16)
    w["vn_W1"] = np.ascontiguousarray(
        np.asarray(inputs["vn_W1"], np.float32).transpose(1, 0, 2)
        .reshape(D, (L - 1) * D))
    w["vn_W2"] = np.ascontiguousarray(
        np.asarray(inputs["vn_W2"], np.float32).transpose(1, 0, 2)
        .reshape(D, (L - 1) * D))
    iota = np.broadcast_to(np.arange(P, dtype=np.float32), (P, P))
    w["iota_row"] = np.ascontiguousarray(iota).astype(NP_BF16)
    ln = np.asarray(inputs["norm_g"], np.float32).reshape(-1)
    w["grep"] = np.broadcast_to(ln, (P, L * D)).copy()
    lb = np.asarray(inputs["norm_b"], np.float32).reshape(-1)
    w["brep"] = np.broadcast_to(lb, (P, L * D)).copy()
    gb = np.asarray(inputs["gcn_b"], np.float32).reshape(-1)
    w["gbrep"] = np.broadcast_to(gb, (P, L * D)).copy()
    vnr = np.asarray(inputs["vn_emb"], np.float32).reshape(-1)
    w["vnrep"] = np.broadcast_to(vnr, (P, D)).copy()
    return w


def build_core_inputs(cfg, core, shared, inputs):
    m = dict(shared)
    m.update({k: core[k] for k in ("idxA", "idxB", "ety24", "doff")})
    nr = cfg.NLOC_REAL
    bb = np.full((cfg.NLOC,), -1.0, np.float32)
    bb[:nr] = core["batch_loc"].astype(np.float32)
    bcols = bb.reshape(cfg.NBLK, P).T
    m["batch0"] = bcols.astype(np.float32)
    m["batch1"] = (bcols - 128.0).astype(np.float32)
    oh = np.zeros((cfg.GT, cfg.NBLK, P, P), NP_FP8)
    bi = bb.astype(np.int64)
    for t in range(cfg.NBLK):
        for p in range(P):
            g = bi[t * P + p]
            if g >= 0:
                oh[g // P, t, g % P, p] = 1.0
    m["oh_bat"] = oh
    # atom one-hots: [kch, NBLK, P(k), P(n)] for this core's nodes
    x = np.asarray(inputs["x"], np.int64)
    nf = x.shape[1]
    nv = np.asarray(inputs["atom_emb"]).shape[1]
    kch = -(-nf * nv // P)
    xs = np.zeros((cfg.NLOC, nf), np.int64)
    cid = int(core["cid"])
    xs[:nr] = x[cid * nr:(cid + 1) * nr]
    kv = (np.arange(nf) * nv)[None, :] + xs          # [NLOC, nf] in [0, nf*nv)
    ohx = np.zeros((kch, cfg.NBLK, P, P), NP_FP8)
    tt = np.arange(cfg.NLOC) // P
    pp = np.arange(cfg.NLOC) % P
    for f in range(nf):
        k = kv[:, f]
        valid = np.zeros(cfg.NLOC, bool)
        valid[:nr] = True
        ohx[k[valid] // P, tt[valid], k[valid] % P, pp[valid]] = 1.0
    m["atom_oh"] = np.ascontiguousarray(ohx.transpose(1, 2, 0, 3))  # [NBLK,P(k),kch,P(n)]
    return m


# ---------------- the SPMD bass program ----------------

def build_bass(cfg, w):
    nc = bacc.Bacc("TRN2", target_bir_lowering=False, debug=False,
                   num_devices=NC_)
    NBLK, TBH = cfg.NBLK, cfg.TBLK_H
    NTILE, NTILE_H = cfg.NTILE, cfg.NTILE_H
    NLOC, NPAD, HALF, GT = cfg.NLOC, cfg.NPAD, cfg.HALF, cfg.GT
    KCH = w["atom_tab"].shape[1]

    ln_triv = np.allclose(w["grep"], 1) and np.allclose(w["brep"], 0)
    gb_triv = np.allclose(w["gbrep"], 0)
    vn_triv = np.allclose(w["vnrep"], 0)

    def din(name, arr_shape, dt):
        return nc.dram_tensor(name, list(arr_shape), dt, kind="ExternalInput")

    atom_oh = din("atom_oh", [NBLK, P, KCH, P], FP8)
    atom_tab = din("atom_tab", [P, KCH, D], BF16)
    bond_tab = din("bond_tab", [24, D], BF16)
    viter_d = din("viter", [24, 1], F32)
    ety24_d = din("ety24", [24, 2 * cfg.SLOTS_H], FP8)
    idxA_d = din("idxA", [P, cfg.SLOTS_H // 16], I16)
    idxB_d = din("idxB", [P, cfg.SLOTS_H // 16], I16)
    doff_d = din("doff", [P, NTILE], F32)
    batch0_d = din("batch0", [P, NBLK], F32)
    batch1_d = din("batch1", [P, NBLK], F32)
    oh_bat_d = din("oh_bat", [GT, NBLK, P, P], FP8)
    gcnW_d = din("gcn_W", [D, L * D], FP16)
    vnW1_d = din("vn_W1", [D, (L - 1) * D], F32)
    vnW2_d = din("vn_W2", [D, (L - 1) * D], F32)
    iota_d = din("iota_row", [P, P], BF16)
    grep_d = din("grep", [P, L * D], F32)
    brep_d = din("brep", [P, L * D], F32)
    gbrep_d = din("gbrep", [P, L * D], F32)
    vnrep_d = din("vnrep", [P, D], F32)
    out_p = nc.dram_tensor("out", [NLOC, D], F32, kind="ExternalOutput")

    # h feed table: [NPAD, D] fp16 (+2 pad rows for the odd-pair view);
    # AllGather writes it directly (Shared scratchpad).
    hfull = nc.dram_tensor("hfull", [NPAD + 2, D], FP16, addr_space="Shared")
    shard_b = nc.dram_tensor("shard_b", [NLOC, D], FP16)
    vt_in = nc.dram_tensor("vt_in", [GT * P, D], F32)
    vt_out = nc.dram_tensor("vt_out", [GT * P, D], F32)
    RG = [list(range(NC_))]
    A = mybir.AluOpType
    AF = mybir.ActivationFunctionType

    with tile.TileContext(nc) as tc:
        with tc.tile_pool(name="res", bufs=1) as res, \
             tc.tile_pool(name="wk", bufs=2) as wk, \
             tc.tile_pool(name="gat", bufs=2) as gat, \
             tc.tile_pool(name="psA", bufs=2, space="PSUM") as psA, \
             tc.tile_pool(name="psV", bufs=1, space="PSUM") as psV, \
             tc.tile_pool(name="psT", bufs=1, space="PSUM") as psT, \
             tc.tile_pool(name="psM", bufs=2, space="PSUM") as psM:

            ST = res.tile([P, NTILE, P], FP8, tag="ST")
            EM = res.tile([P, NTILE, D], FP16 if EM_DT_ENV else FP8, tag="EM")
            IOTA = res.tile([P, P], BF16, tag="IOTA")
            IDENT = res.tile([P, P], F32, tag="IDENT")
            HRES = res.tile([P, NBLK, D], F32, tag="HRES")
            HFEED = res.tile([P, NBLK, D], FP16, tag="HFEED")
            H2 = res.tile([P, NBLK, D], FP16, tag="H2")
            OHT = res.tile([P, GT * NBLK, P], FP8, tag="OHT")
            OHBT = res.tile([P, GT * NBLK, P], FP8, tag="OHBT")
            VNT = res.tile([P, GT, D], F32, tag="VNT")
            VNT16 = res.tile([P, GT, D], FP16, tag="VNT16")
            GW = res.tile([D, L * D], FP16, tag="GW")
            VW1 = res.tile([D, (L - 1) * D], F32, tag="VW1")
            VW2 = res.tile([D, (L - 1) * D], F32, tag="VW2")
            MU = res.tile([P, NBLK], F32, tag="MU")
            SSQ = res.tile([P, NBLK], F32, tag="SSQ")
            RS = res.tile([P, NBLK], F32, tag="RS")
            GREP = BREP = GBREP = VNREP = None
            if not ln_triv:
                GREP = res.tile([P, L * D], F32, tag="GREP")
                BREP = res.tile([P, L * D], F32, tag="BREP")
            if not gb_triv:
                GBREP = res.tile([P, L * D], F32, tag="GBREP")
            if not vn_triv:
                VNREP = res.tile([P, D], F32, tag="VNREP")

            nc.sync.dma_start(out=IOTA[:], in_=iota_d[:])
            make_identity(nc, IDENT[:])
            nc.sync.dma_start(out=GW[:], in_=gcnW_d[:])
            nc.sync.dma_start(out=VW1[:], in_=vnW1_d[:])
            nc.sync.dma_start(out=VW2[:], in_=vnW2_d[:])
            nc.sync.dma_start(
                out=OHBT[:], in_=oh_bat_d[:].rearrange("q t p c -> p (q t) c"))
            if not ln_triv:
                nc.sync.dma_start(out=GREP[:], in_=grep_d[:])
                nc.sync.dma_start(out=BREP[:], in_=brep_d[:])
            if not gb_triv:
                nc.sync.dma_start(out=GBREP[:], in_=gbrep_d[:])
            if not vn_triv:
                nc.sync.dma_start(out=VNREP[:], in_=vnrep_d[:])

            # zero the 2 pad rows past the AllGather output (read by the
            # odd-pair gather view)
            ZT = res.tile([P, D], FP16, tag="ZT")
            nc.vector.memset(ZT[:], 0.0)
            nc.sync.dma_start(out=hfull[NPAD:NPAD + 2, :], in_=ZT[0:2, :])
            BIX = res.tile([P, 1], I16, tag="BIX")
            nc.vector.memset(BIX[:], 0)
            BBUF = res.tile([P, 1, 2 * D], FP16, tag="BBUF")

            # ----- atom embeddings -> h0, feed0 -----
            with tc.tile_pool(name="at", bufs=2) as at:
                atab = at.tile([P, KCH, D], BF16, tag="atab")
                nc.sync.dma_start(out=atab[:], in_=atom_tab[:])
                for t in range(NBLK):
                    ohx = at.tile([P, KCH, P], FP8, tag="ohx")
                    nc.sync.dma_start(out=ohx[:], in_=atom_oh[t])
                    ph = psM.tile([P, D], F32, tag="pmm")
                    for k in range(KCH):
                        nc.tensor.matmul(out=ph[:], lhsT=ohx[:, k, :],
                                         rhs=atab[:, k, :],
                                         start=(k == 0), stop=(k == KCH - 1))
                    if vn_triv:
                        nc.vector.tensor_copy(out=HRES[:, t, :], in_=ph[:])
                    else:
                        nc.vector.tensor_tensor(out=HRES[:, t, :], in0=ph[:],
                                                in1=VNREP[:], op=A.add)
                    nc.vector.tensor_copy(out=HFEED[:, t, :], in_=HRES[:, t, :])

            # ----- helpers -----
            def ln_relu(dst, src_ap, li, relu):
                mu = wk.tile([P, 1], F32, tag="mu")
                nc.vector.tensor_reduce(out=mu[:], in_=src_ap, op=A.add,
                                        axis=mybir.AxisListType.X)
                nc.vector.tensor_scalar(out=mu[:], in0=mu[:], scalar1=1.0 / D,
                                        scalar2=None, op0=A.mult)
                dt_ = wk.tile([P, D], F32, tag="lnd")
                nc.vector.tensor_scalar(out=dt_[:], in0=src_ap, scalar1=mu[:],
                                        scalar2=None, op0=A.subtract)
                jk = wk.tile([P, D], F32, tag="lnj")
                ssq = wk.tile([P, 1], F32, tag="ssq")
                nc.scalar.activation(out=jk[:], in_=dt_[:], func=AF.Square,
                                     accum_out=ssq[:])
                nc.vector.tensor_scalar(out=ssq[:], in0=ssq[:], scalar1=1.0 / D,
                                        scalar2=LN_EPS, op0=A.mult, op1=A.add)
                nc.scalar.sqrt(out=ssq[:], in_=ssq[:])
                rs = wk.tile([P, 1], F32, tag="rs")
                nc.vector.reciprocal(out=rs[:], in_=ssq[:])
                if ln_triv or li is None:
                    if relu:
                        nc.vector.tensor_scalar(
                            out=dst, in0=dt_[:], scalar1=rs[:], scalar2=0.0,
                            op0=A.mult, op1=A.max)
                    else:
                        nc.vector.tensor_scalar(
                            out=dst, in0=dt_[:], scalar1=rs[:], scalar2=None,
                            op0=A.mult)
                else:
                    t1 = wk.tile([P, D], F32, tag="lnt1")
                    nc.vector.tensor_scalar(out=t1[:], in0=dt_[:], scalar1=rs[:],
                                            scalar2=None, op0=A.mult)
                    t2 = wk.tile([P, D], F32, tag="lnt2")
                    nc.vector.tensor_tensor(out=t2[:], in0=t1[:],
                                            in1=GREP[:, li * D:(li + 1) * D],
                                            op=A.mult)
                    t3 = wk.tile([P, D], F32, tag="lnt3")
                    nc.vector.tensor_tensor(out=t3[:], in0=t2[:],
                                            in1=BREP[:, li * D:(li + 1) * D],
                                            op=A.add)
                    if relu:
                        nc.vector.tensor_scalar(out=dst, in0=t3[:], scalar1=0.0,
                                                scalar2=None, op0=A.max)
                    else:
                        nc.vector.tensor_copy(out=dst, in_=t3[:])

            def write_shard_allgather(prev_gathers):
                sh_bi = nc.sync.dma_start(
                    out=shard_b[:].rearrange("(a p) d -> p a d", p=P),
                    in_=HFEED[:])
                ag = nc.gpsimd.collective_compute(
                    "AllGather", A.bypass, replica_groups=RG,
                    ins=[shard_b[:]], outs=[hfull[0:NPAD, :]])
                for gprev in prev_gathers:
                    add_dep_helper(ag.ins, gprev.ins, reason="AG after gathers")
                return ag

            # pair views over hfull for the parity gathers: stride 2 rows
            # (256B), element = 2 rows (256B); wanted row is the first half.
            _hap = hfull[:]
            evenv = dataclasses.replace(
                _hap, ap=type(_hap.ap)([[2 * D, HALF], [1, 2 * D]]))
            oddv = dataclasses.replace(evenv, offset=D)
            dma_sem = nc.alloc_semaphore("swdge_pref")
            PF = min(int(os.environ.get("K_PF", "0")), cfg.NCHUNK)
            sem_cum = [0]
            last_wt = [None]

            def prep_chunk(ch):
                """prepare_only gathers for chunk ch (descriptors written now,
                DMA fired by the next trigger_dma; hfull dep defers there)."""
                bufA = gat.tile([P, cfg.CH_TILES, 2 * D], FP16, tag="gA")
                bufB = gat.tile([P, cfg.CH_TILES, 2 * D], FP16, tag="gB")
                c0 = ch * cfg.CH_IDX // 16
                ixA = gat.tile([P, cfg.CH_IDX // 16], I16, tag="ixA")
                ixB = gat.tile([P, cfg.CH_IDX // 16], I16, tag="ixB")
                dA = nc.sync.dma_start(out=ixA[:],
                                       in_=idxA_d[:, c0:c0 + cfg.CH_IDX // 16])
                dB = nc.sync.dma_start(out=ixB[:],
                                       in_=idxB_d[:, c0:c0 + cfg.CH_IDX // 16])
                if last_wt[0] is not None:
                    add_dep_helper(dA.ins, last_wt[0].ins,
                                   reason="ix reuse after drains")
                    add_dep_helper(dB.ins, last_wt[0].ins,
                                   reason="ix reuse after drains")
                pA = nc.gpsimd.dma_gather(
                    out_ap=bufA[:], in_ap=evenv, idxs_ap=ixA[:],
                    num_idxs=cfg.CH_IDX, num_idxs_reg=cfg.CH_IDX,
                    elem_size=2 * D, single_packet=False,
                    prepare_only=True, sem=dma_sem)
                pB = nc.gpsimd.dma_gather(
                    out_ap=bufB[:], in_ap=oddv, idxs_ap=ixB[:],
                    num_idxs=cfg.CH_IDX, num_idxs_reg=cfg.CH_IDX,
                    elem_size=2 * D, single_packet=False,
                    prepare_only=True, sem=dma_sem)
                return (bufA, bufB, pA, pB)

            def edge_phase(l, ag_bi, fuse_stats, prefetched=()):
                gathers = []
                Wl = GW[:, l * D:(l + 1) * D]
                trigger = None
                wt = None
                if prefetched:
                    # gate the trigger behind the AG through a Pool engine
                    # nop: deps on the nop materialize as a Collectives-sem
                    # wait, and the trigger's engine-lane wait covers the nop
                    # (a direct collective dep on InstTriggerDma is dropped).
                    xnop = nc.gpsimd.engine_nop()
                    if ag_bi is not None:
                        add_dep_helper(xnop.ins, ag_bi.ins, reason="AG gate")
                    trigger = nc.gpsimd.trigger_dma(count=None)
                    add_dep_helper(trigger.ins, xnop.ins,
                                   reason="trigger after AG gate")
                    # sim-contract wait: each prep incs the sem by 16 at
                    # drain (HW granularity differs; the barrier gather below
                    # provides the real ring-FIFO ordering there).
                    sem_cum[0] += 16 * 2 * len(prefetched)
                    wts = nc.gpsimd.wait_ge(dma_sem, sem_cum[0])
                    add_dep_helper(wts.ins, trigger.ins,
                                   reason="sem wait after trigger")
                    # barrier gather: the SWDGE ring drains FIFO per engine,
                    # so this gather's completion implies every triggered
                    # prefetch drain has landed.
                    wt = nc.gpsimd.dma_gather(
                        out_ap=BBUF[:], in_ap=evenv, idxs_ap=BIX[:],
                        num_idxs=16, num_idxs_reg=16, elem_size=2 * D,
                        single_packet=False)
                    add_dep_helper(wt.ins, wts.ins,
                                   reason="ring barrier after sem wait")
                    last_wt[0] = wt
                    gathers.append(trigger)
                    gathers.append(wt)
                    for (_, _, pA, pB) in prefetched:
                        gathers += [pA, pB]
                for ch in range(cfg.NCHUNK):
                    if ch < len(prefetched):
                        bufA, bufB = prefetched[ch][0], prefetched[ch][1]
                    else:
                        bufA = gat.tile([P, cfg.CH_TILES, 2 * D], FP16,
                                        tag="gA")
                        bufB = gat.tile([P, cfg.CH_TILES, 2 * D], FP16,
                                        tag="gB")
                        c0 = ch * cfg.CH_IDX // 16
                        ixA = gat.tile([P, cfg.CH_IDX // 16], I16, tag="ixA")
                        ixB = gat.tile([P, cfg.CH_IDX // 16], I16, tag="ixB")
                        nc.sync.dma_start(
                            out=ixA[:], in_=idxA_d[:, c0:c0 + cfg.CH_IDX // 16])
                        nc.sync.dma_start(
                            out=ixB[:], in_=idxB_d[:, c0:c0 + cfg.CH_IDX // 16])
                        gA = nc.gpsimd.dma_gather(
                            out_ap=bufA[:], in_ap=evenv, idxs_ap=ixA[:],
                            num_idxs=cfg.CH_IDX, num_idxs_reg=cfg.CH_IDX,
                            elem_size=2 * D, single_packet=False)
                        gB = nc.gpsimd.dma_gather(
                            out_ap=bufB[:], in_ap=oddv, idxs_ap=ixB[:],
                            num_idxs=cfg.CH_IDX, num_idxs_reg=cfg.CH_IDX,
                            elem_size=2 * D, single_packet=False)
                        if ag_bi is not None:
                            add_dep_helper(gA.ins, ag_bi.ins,
                                           reason="gather after AG")
                            add_dep_helper(gB.ins, ag_bi.ins,
                                           reason="gather after AG")
                        if trigger is not None:
                            add_dep_helper(gA.ins, trigger.ins,
                                           reason="ring order after trigger")
                            add_dep_helper(gB.ins, trigger.ins,
                                           reason="ring order after trigger")
                        gathers += [gA, gB]
                    rhs2 = []
                    for half, buf in ((0, bufA), (1, bufB)):
                        # tt lives in the gathered buffer's spare half
                        tt = buf[:, :, D:2 * D]
                        rhs = wk.tile([P, cfg.CH_TILES, 2 * D], FP16,
                                      tag=f"rhs{half}")
                        uu = rhs[:, :, 0:D]
                        em_sl = EM[:, half * NTILE_H + ch * cfg.CH_TILES:
                                   half * NTILE_H + (ch + 1) * cfg.CH_TILES, :]
                        ttadd = nc.vector.tensor_tensor(
                            out=tt, in0=buf[:, :, 0:D], in1=em_sl, op=A.add)
                        if ch < len(prefetched) and wt is not None:
                            add_dep_helper(ttadd.ins, wt.ins,
                                           reason="read after prefetch drain")
                        nc.scalar.activation(out=uu, in_=tt, func=AF.Exp)
                        nc.vector.scalar_tensor_tensor(
                            out=rhs[:, :, D:2 * D], in0=tt, scalar=0.0,
                            in1=uu, op0=A.max, op1=A.mult)
                        nc.vector.tensor_scalar(out=uu, in0=uu,
                                                scalar1=1.0, scalar2=None,
                                                op0=A.max)
                        rhs2.append(rhs)
                    for bb in range(cfg.CHUNK_BLKS):
                        b = ch * cfg.CHUNK_BLKS + bb
                        pb = psA.tile([P, 2 * D], F32, tag="blk")
                        for half in (0, 1):
                            for j in range(TBH):
                                gt_id = half * NTILE_H + b * TBH + j
                                nc.tensor.matmul(
                                    out=pb[:], lhsT=ST[:, gt_id, :],
                                    rhs=rhs2[half][:, bb * TBH + j, :],
                                    start=(half == 0 and j == 0),
                                    stop=(half == 1 and j == TBH - 1))
                        dmx = wk.tile([P, D], F32, tag="dmx")
                        nc.vector.tensor_scalar(out=dmx[:], in0=pb[:, 0:D],
                                                scalar1=1e-16, scalar2=None,
                                                op0=A.max)
                        rcp = wk.tile([P, D], F32, tag="rcp")
                        nc.vector.reciprocal(out=rcp[:], in_=dmx[:])
                        mlpin = wk.tile([P, D], F32, tag="mlpin")
                        nc.vector.tensor_tensor(out=mlpin[:], in0=pb[:, D:2 * D],
                                                in1=rcp[:], op=A.mult)
                        nc.vector.scalar_tensor_tensor(
                            out=mlpin[:], in0=mlpin[:], scalar=MSG_EPS,
                            in1=HFEED[:, b, :], op0=A.add, op1=A.add)
                        pxt = psT.tile([D, P], F32, tag="pxt")
                        nc.tensor.transpose(out=pxt[:], in_=mlpin[:],
                                            identity=IDENT[:])
                        xt = wk.tile([D, P], FP16, tag="xt")
                        nc.scalar.copy(out=xt[:], in_=pxt[:])
                        ph2 = psM.tile([P, D], F32, tag="pmm")
                        nc.tensor.matmul(out=ph2[:], lhsT=xt[:], rhs=Wl,
                                         start=True, stop=True)
                        if l == 0 and gb_triv:
                            nc.vector.tensor_copy(out=HRES[:, b, :], in_=ph2[:])
                        elif l == 0:
                            nc.vector.tensor_tensor(
                                out=HRES[:, b, :], in0=ph2[:],
                                in1=GBREP[:, l * D:(l + 1) * D], op=A.add)
                        else:
                            nc.vector.tensor_tensor(out=HRES[:, b, :],
                                                    in0=ph2[:],
                                                    in1=HRES[:, b, :], op=A.add)
                            if not gb_triv:
                                nc.vector.tensor_tensor(
                                    out=HRES[:, b, :], in0=HRES[:, b, :],
                                    in1=GBREP[:, l * D:(l + 1) * D], op=A.add)
                        if fuse_stats:
                            # LN row stats of HRES[b] (sum and sum-of-squares)
                            # computed inline; sqrt/reciprocal batched later.
                            nc.vector.tensor_reduce(
                                out=MU[:, b:b + 1], in_=HRES[:, b, :],
                                op=A.add, axis=mybir.AxisListType.X)
                            sqt = wk.tile([P, D], F32, tag="sqt")
                            nc.vector.tensor_tensor(out=sqt[:],
                                                    in0=HRES[:, b, :],
                                                    in1=HRES[:, b, :],
                                                    op=A.mult)
                            nc.vector.tensor_reduce(
                                out=SSQ[:, b:b + 1], in_=sqt[:],
                                op=A.add, axis=mybir.AxisListType.X)
                return gathers

            def batch_ln_stats():
                # MU <- mean, RS <- 1/sqrt(var+eps), one [P, NBLK] pass
                nc.vector.tensor_scalar(out=MU[:], in0=MU[:], scalar1=1.0 / D,
                                        scalar2=None, op0=A.mult)
                msq = wk.tile([P, NBLK], F32, tag="bmsq")
                nc.vector.tensor_tensor(out=msq[:], in0=MU[:], in1=MU[:],
                                        op=A.mult)
                var = wk.tile([P, NBLK], F32, tag="bvar")
                nc.vector.tensor_scalar(out=var[:], in0=SSQ[:],
                                        scalar1=1.0 / D, scalar2=None,
                                        op0=A.mult)
                nc.vector.tensor_tensor(out=var[:], in0=var[:], in1=msq[:],
                                        op=A.subtract)
                # clamp before +eps: one-pass var can go slightly negative
                nc.vector.tensor_scalar(out=var[:], in0=var[:], scalar1=0.0,
                                        scalar2=LN_EPS, op0=A.max, op1=A.add)
                sq = wk.tile([P, NBLK], F32, tag="bsq")
                nc.scalar.sqrt(out=sq[:], in_=var[:])
                nc.vector.reciprocal(out=RS[:], in_=sq[:])

            # ===== layer 0 =====
            pref = [prep_chunk(ch) for ch in range(PF)]
            ag = write_shard_allgather([])
            B0 = res.tile([P, NBLK], F32, tag="B0")
            B1 = res.tile([P, NBLK], F32, tag="B1")
            nc.sync.dma_start(out=B0[:], in_=batch0_d[:])
            nc.sync.dma_start(out=B1[:], in_=batch1_d[:])
            # ----- one-hot builds + bond EM matmuls (upfront) -----
            with tc.tile_pool(name="su", bufs=2) as su:
                DOFF = su.tile([P, NTILE], F32, tag="DOFF", bufs=1)
                pass
                nc.sync.dma_start(out=DOFF[:], in_=doff_d[:])
                for t in range(NTILE):
                    nc.vector.tensor_scalar(out=ST[:, t, :], in0=IOTA[:],
                                            scalar1=DOFF[:, t:t + 1],
                                            scalar2=None, op0=A.is_equal)
                BT24 = su.tile([24, D], BF16, tag="bt24", bufs=1)
                nc.sync.dma_start(out=BT24[:], in_=bond_tab[:])
                VIT = su.tile([24, 1], F32, tag="vit", bufs=1)
                nc.sync.dma_start(out=VIT[:], in_=viter_d[:])
                ECH = 16
                for c0 in range(0, NTILE, ECH):
                    nch = min(ECH, NTILE - c0)
                    ET = su.tile([24, ECH * P], FP8, tag="et")
                    nc.sync.dma_start(out=ET[:, 0:nch * P],
                                      in_=ety24_d[:, c0 * P:(c0 + nch) * P])
                    for j in range(nch):
                        OH24 = su.tile([24, P], BF16, tag="oh24")
                        nc.vector.tensor_scalar(
                            out=OH24[:], in0=ET[:, j * P:(j + 1) * P],
                            scalar1=VIT[:], scalar2=None, op0=A.is_equal)
                        pem = psM.tile([P, D], F32, tag="pmm")
                        nc.tensor.matmul(out=pem[:], lhsT=OH24[:], rhs=BT24[:],
                                         start=True, stop=True)
                        nc.scalar.copy(out=EM[:, c0 + j, :], in_=pem[:])

            gathers = edge_phase(0, ag, fuse_stats=ln_triv, prefetched=pref)
            # OHT one-hots are first needed at the post phase of layer 1;
            # emit after the layer-0 chunks so DVE slack absorbs them.
            for q in range(GT):
                bq = B0 if q == 0 else B1
                for t in range(NBLK):
                    nc.vector.tensor_scalar(
                        out=OHT[:, q * NBLK + t, :], in0=IOTA[:],
                        scalar1=bq[:, t:t + 1], scalar2=None,
                        op0=A.is_equal)

            # ===== layers 1..L-1 =====
            for l in range(1, L):
                pvt = []
                for q in range(GT):
                    pvtq = psV.tile([P, D], F32, tag=f"vt{q}", name=f"pvt{q}")
                    pvt.append(pvtq)
                if ln_triv:
                    batch_ln_stats()
                for t in range(NBLK):
                    if ln_triv:
                        h2t = wk.tile([P, D], F32, tag="h2t")
                        nc.vector.tensor_scalar(
                            out=h2t[:], in0=HRES[:, t, :],
                            scalar1=MU[:, t:t + 1], scalar2=RS[:, t:t + 1],
                            op0=A.subtract, op1=A.mult)
                        nc.vector.tensor_scalar(
                            out=H2[:, t, :], in0=h2t[:], scalar1=0.0,
                            scalar2=None, op0=A.max)
                    else:
                        ln_relu(H2[:, t, :], HRES[:, t, :], l - 1, True)
                    for q in range(GT):
                        nc.tensor.matmul(out=pvt[q][:],
                                         lhsT=OHT[:, q * NBLK + t, :],
                                         rhs=H2[:, t, :], start=(t == 0),
                                         stop=(t == NBLK - 1),
                                         skip_group_check=True)
                vtl = wk.tile([P, GT, D], F32, tag="vtl")
                for q in range(GT):
                    if l == 1 and vn_triv:
                        nc.vector.tensor_copy(out=vtl[:, q, :], in_=pvt[q][:])
                    elif l == 1:
                        nc.vector.tensor_tensor(out=vtl[:, q, :], in0=pvt[q][:],
                                                in1=VNREP[:], op=A.add)
                    else:
                        nc.vector.tensor_tensor(out=vtl[:, q, :], in0=pvt[q][:],
                                                in1=VNT[:, q, :], op=A.add)
                nc.sync.dma_start(
                    out=vt_in[:].rearrange("(a p) d -> p a d", p=P), in_=vtl[:])
                ar = nc.gpsimd.collective_compute(
                    "AllReduce", A.add, replica_groups=RG,
                    ins=[vt_in[:]], outs=[vt_out[:]])
                vtr = wk.tile([P, GT, D], F32, tag="vtr")
                r_bi = nc.sync.dma_start(
                    out=vtr[:], in_=vt_out[:].rearrange("(a p) d -> p a d", p=P))
                add_dep_helper(r_bi.ins, ar.ins, reason="read after AR")
                # prep next layer's first chunks while Pool idles through
                # the AR/vn-MLP/AllGather window
                pref = [prep_chunk(ch) for ch in range(PF)]

                def vn_mlp(src_t, Wsl, dst_f32, dst_f16):
                    for q in range(GT):
                        pxt = psT.tile([D, P], F32, tag="pxt")
                        nc.tensor.transpose(out=pxt[:], in_=src_t[:, q, :],
                                            identity=IDENT[:])
                        xt = wk.tile([D, P], F32, tag="xtf")
                        nc.scalar.copy(out=xt[:], in_=pxt[:])
                        pu = psM.tile([P, D], F32, tag="pmm")
                        nc.tensor.matmul(out=pu[:], lhsT=xt[:], rhs=Wsl,
                                         start=True, stop=True)
                        uf = wk.tile([P, D], F32, tag="uf")
                        nc.vector.tensor_copy(out=uf[:], in_=pu[:])
                        ln_relu(dst_f32[:, q, :], uf[:], None, True)
                        if dst_f16 is not None:
                            nc.vector.tensor_copy(out=dst_f16[:, q, :],
                                                  in_=dst_f32[:, q, :])

                u1 = wk.tile([P, GT, D], F32, tag="u1")
                vn_mlp(vtr, VW1[:, (l - 1) * D:l * D], u1, None)
                vn_mlp(u1, VW2[:, (l - 1) * D:l * D], VNT, VNT16)

                for t in range(NBLK):
                    pv = psM.tile([P, D], F32, tag="pmm")
                    for q in range(GT):
                        nc.tensor.matmul(out=pv[:],
                                         lhsT=OHBT[:, q * NBLK + t, :],
                                         rhs=VNT16[:, q, :], start=(q == 0),
                                         stop=(q == GT - 1))
                    nc.vector.tensor_tensor(out=HFEED[:, t, :],
                                            in0=H2[:, t, :], in1=pv[:],
                                            op=A.add)
                ag = write_shard_allgather(gathers)
                gathers = edge_phase(l, ag, fuse_stats=ln_triv,
                                     prefetched=pref)

            # ===== output layernorm =====
            if ln_triv:
                batch_ln_stats()
                for t in range(NBLK):
                    ot = wk.tile([P, D], F32, tag="ot")
                    nc.vector.tensor_scalar(
                        out=ot[:], in0=HRES[:, t, :], scalar1=MU[:, t:t + 1],
                        scalar2=RS[:, t:t + 1], op0=A.subtract, op1=A.mult)
                    nc.sync.dma_start(out=out_p[t * P:(t + 1) * P, :],
                                      in_=ot[:])
            else:
                for t in range(NBLK):
                    ot = wk.tile([P, D], F32, tag="ot")
                    ln_relu(ot[:], HRES[:, t, :], L - 1, False)
                    nc.sync.dma_start(out=out_p[t * P:(t + 1) * P, :],
                                      in_=ot[:])

    nc.compile()
    return nc


# ---------------- driver ----------------

_CACHE = {}


def run_cfg(cfg, inputs, trace=False):
    key = (cfg.N, cfg.E, cfg.G, cfg.NBLK, cfg.TBLK_H)
    cores = build_layout(cfg, inputs["edge_index"], inputs["edge_attr"],
                         inputs["batch"])
    for c in range(NC_):
        cores[c]["cid"] = c
    shared = build_shared_inputs(cfg, inputs)
    if key not in _CACHE:
        _CACHE[key] = build_bass(cfg, shared)
    nc = _CACHE[key]
    in_maps = [build_core_inputs(cfg, cores[c], shared, inputs)
               for c in range(NC_)]
    if os.environ.get("K_SIM", "0") == "1":
        from concourse.bass_interp import MultiCoreSim
        sim = MultiCoreSim(nc, num_cores=NC_, require_finite=False,
                           require_nnan=False)
        for c, cs in enumerate(sim.cores.values()):
            for k, v in in_maps[c].items():
                cs.tensor(k)[:] = v
        sim.simulate(check_with_hw=False)
        nr = cfg.NLOC_REAL
        outp = np.zeros((cfg.N, D), np.float32)
        for c, cs in enumerate(sim.cores.values()):
            outp[c * nr:(c + 1) * nr] = np.asarray(cs.tensor("out"))[:nr]
        return outp, None
    import importlib.util as _ilu
    hook_py = "/opt/trn_rl_repo/antenv/axon_hooks.py"
    if trace and os.path.exists(hook_py) and "antenv.axon_hooks" not in sys.modules:
        try:
            _spec = _ilu.spec_from_file_location("antenv.axon_hooks", hook_py)
            _mod = _ilu.module_from_spec(_spec)
            _spec.loader.exec_module(_mod)
            sys.modules["antenv.axon_hooks"] = _mod
        except Exception:
            trace = False
    from concourse.bass_utils import run_bass_kernel_spmd
    res = run_bass_kernel_spmd(nc, in_maps, list(range(NC_)), trace=trace)
    nr = cfg.NLOC_REAL
    outp = np.zeros((cfg.N, D), np.float32)
    for c in range(NC_):
        outp[c * nr:(c + 1) * nr] = res.results[c]["out"][:nr]
    return outp, res


def kernel(**inputs):
    cfg = CFG.full()
    out, _ = run_cfg(cfg, inputs, trace=False)
    return out

